# revision 62
# baseline (speedup 1.0000x reference)
"""Multi-head attention (B=4, N=2048, D=768, H=12, Dh=64) on 8 TRN2 NeuronCores.

Sharding: core c -> batch b = c//2, query rows half = c%2 (1024 rows each).
Each core computes all 12 heads for its (batch, query-half) against the full
2048-key sequence, so outputs are disjoint and no collective is needed.

Head-sequential eager pipeline: the kernel is a continuous stream of
"half-phases" (one per head). In half h, step k emits:
  - scores(h, k): two row-tiled K=64 matmuls (the head's 64 q/k dims live in
    array rows hp*64) -> S^T[128 keys, 1024 q] in a psS generation,
  - exp(h, k) on ACT (or, for k in DVE_K, a 2-instruction custom DVE op
    computing (1 + t + t^2/2)^2048 by repeated squaring),
  - attn@V of the PREVIOUS half at tile k: two [K=128, M=128, N=512] matmuls
    against the V panel (65 cols per head: 64 dims + ones column whose output
    row 64 is the softmax denominator),
  - one projection/V-panel filler step (next pair's Q^T/K^T, V panels).
The exp stream never waits on a serial prologue; the scalar engine runs
near-continuously.  Denominator rows are normalized baseline-style: fast
reciprocal + two K<=1 selector matmuls broadcast 1/d to the head's partition
rows, then one multiply.
"""

import numpy as np

import concourse.bass as bass
import concourse.bacc as bacc
import concourse.mybir as mybir
import concourse.tile as tile
from concourse.bass_utils import run_bass_kernel_spmd

N_CORES = 8
B, N, D = 4, 2048, 768
H, DH = 12, 64
NQ = 1024           # query rows per core
COLS = 3 * D        # 2304 qkv columns
DT = D // 128       # 6 partition tiles of the model dim
NT = N // 128       # 16 key tiles
QT_TILES = NQ // 128
NP = DT             # 6 head pairs
VG = DH + 1         # 65: head group width in V (64 cols + ones)

F32 = mybir.dt.float32
BF16 = mybir.dt.bfloat16

# k-tiles whose exp runs on the DVE via the custom squaring ops
DVE_K = (5, 11)


# ---------------- custom DVE exp ----------------
# exp(0.125*s) = u^2048, u = 1 + t + t^2/2 = ((t+1)^2 + 1)/2, t = 0.125*s/2048
def _register_exp_ops():
    import concourse.dve_ops as dve_ops
    from concourse.dve_spec import Spec, Src0, C0, C1, One, sq, lower
    from concourse.dve_uop import DveOpSpec
    from concourse.dve_table_gen import dve_ver_for

    def by_name(name):
        for o in dve_ops.OPS:
            if o.name == name:
                return o
        return None

    got = (by_name("EXP2K_BASE_ANT"), by_name("EXP2K_SQ8_ANT"))
    if got[0] is not None:
        return got

    a = sq(Src0 * C0 + One) + One
    body1 = sq(sq(sq(a * C1)))

    def ref1(in0, in1, s0, s1, imm2):
        u = ((in0.astype(np.float64) * s0 + 1.0) ** 2 + 1.0) * s1
        return (u ** 8).astype(np.float32)

    body2 = sq(sq(sq(sq(sq(sq(sq(sq(Src0))))))))

    def ref2(in0, in1, s0, s1, imm2):
        return (in0.astype(np.float64) ** 256).astype(np.float32)

    ver = dve_ver_for("TRN2")
    ops = []
    for name, body, ref in (("EXP2K_BASE_ANT", body1, ref1),
                            ("EXP2K_SQ8_ANT", body2, ref2)):
        spec = Spec(body=body, reference=ref)
        row = max(dve_ops._SUB_OPCODE_FOR_NAME.values()) + 1
        assert row < 0x20
        tmp = DveOpSpec(name=name, opcode=row, uops=lower(spec, ver=ver),
                        rd1_en=False)
        op = dve_ops.DveOp(name, spec, subdim=False,
                           uops_sha={ver: tmp.sha(ver)})
        dve_ops._SUB_OPCODE_FOR_NAME[name] = row
        dve_ops.OPS.append(op)
        dve_ops.CUSTOM_DVE_SPECS[name] = spec
        ops.append(op)
    return tuple(ops)


EXP_BASE, EXP_SQ8 = _register_exp_ops()


def build(debug_taps=False):
    nc = bacc.Bacc("TRN2", target_bir_lowering=False, debug=False,
                   num_devices=N_CORES)

    xT_d = nc.dram_tensor("xT", [D, N], BF16, kind="ExternalInput")
    wqkv_d = nc.dram_tensor("wqkv", [D, COLS], BF16, kind="ExternalInput")
    wout_d = nc.dram_tensor("wout", [D, D], BF16, kind="ExternalInput")
    bias_d = nc.dram_tensor("bias", [128, D], F32, kind="ExternalInput")
    out_d = nc.dram_tensor("out", [NQ, D], F32, kind="ExternalOutput")

    taps = {}
    if debug_taps:
        for name, shape, dt in (("tap_QT0", [128, NQ], BF16),
                                ("tap_KT0", [128, N], BF16),
                                ("tap_PTA4", [128, NQ], BF16),
                                ("tap_PTA5", [128, NQ], BF16)):
            taps[name] = nc.dram_tensor(name, shape, dt,
                                        kind="ExternalOutput")

    with tile.TileContext(nc) as tc:
        with tc.tile_pool(name="persist", bufs=1) as pp, \
             tc.tile_pool(name="small", bufs=2) as smallp, \
             tc.tile_pool(name="outs", bufs=3) as outsp:

            # V with a ones column per head and 63 pad cols so the attn@V
            # stationary can be a full 128-col slice
            V = [pp.tile([128, H * VG + 63], BF16, name=f"V{i}", tag=f"V{i}")
                 for i in range(NT)]
            AOT = [pp.tile([128, NQ], BF16, name=f"AOT{i}", tag=f"AOT{i}")
                   for i in range(NP)]
            WOB = pp.tile([128, DT * D], BF16, name="WOB", tag="WOB")
            BIAS = pp.tile([128, D], F32, name="BIAS", tag="BIAS")
            E1 = pp.tile([1, 128], BF16, name="E1", tag="E1")
            E2 = pp.tile([1, 128], BF16, name="E2", tag="E2")

            nc.gpsimd.memset(E1[:], 0.0)
            nc.gpsimd.memset(E2[:], 0.0)
            nc.gpsimd.memset(E1[0:1, 0:DH], 1.0)
            nc.gpsimd.memset(E2[0:1, DH:128], 1.0)

            def load_tail_weights():
                nc.sync.dma_start(BIAS[:], bias_d.ap())
                nc.sync.dma_start(
                    WOB[:].rearrange("p (a n) -> p a n", n=D),
                    wout_d.ap().rearrange("(a p) n -> p a n", p=128))

            with tc.tile_pool(name="projin", bufs=1) as projin, \
                 tc.tile_pool(name="qk", bufs=1) as qkp, \
                 tc.tile_pool(name="pt", bufs=1) as ptp, \
                 tc.tile_pool(name="scr", bufs=1) as scrp, \
                 tc.tile_pool(name="psA", bufs=2, space="PSUM") as psA, \
                 tc.tile_pool(name="po", bufs=1, space="PSUM") as pop:

                xTC = [projin.tile([128, DT * 512], BF16, name=f"xTC{c}",
                                   tag=f"xTC{c}") for c in range(4)]

                def dma_xtc(c):
                    nc.sync.dma_start(
                        xTC[c][:].rearrange("p (a n) -> p a n", n=512),
                        xT_d.ap()[:, c * 512:(c + 1) * 512].rearrange(
                            "(a p) n -> p a n", p=128))

                dma_xtc(0)

                def xT_ap(d, lo, hi):
                    c, off = divmod(lo, 512)
                    assert (hi - 1) // 512 == c
                    return xTC[c][:, d * 512 + off:d * 512 + off + (hi - lo)]

                # ---- filler steps (V panels + next-pair projections) ----
                v_steps = []

                def make_v_panel(vp):
                    co = 2 * D + vp * 256
                    wvB = projin.tile([128, DT * 256], BF16, name="wvB",
                                      tag="wvB", bufs=1)
                    wv = [wvB[:, d * 256:(d + 1) * 256] for d in range(DT)]
                    nc.sync.dma_start(
                        wvB[:].rearrange("p (a n) -> p a n", n=256),
                        wqkv_d.ap()[:, co:co + 256].rearrange(
                            "(a p) n -> p a n", p=128))

                    def step(t):
                        ps = psA.tile([128, 512], F32, name="psA", tag="psA")
                        for d in range(DT):
                            nc.tensor.matmul(
                                ps[:, :256],
                                xT_ap(d, t * 128, (t + 1) * 128),
                                wv[d],
                                start=(d == 0), stop=(d == DT - 1))
                        dst = V[t][:, 0:H * VG].rearrange(
                            "p (h c) -> p h c", c=VG)
                        nc.vector.tensor_copy(
                            dst[:, vp * 4:(vp + 1) * 4, 0:DH],
                            ps[:, :256].rearrange("p (h c) -> p h c", c=DH))
                        if vp == 0:
                            ones = V[t][:, 0:H * VG].rearrange(
                                "p (h c) -> p h c", c=VG)[:, :, DH:VG]
                            nc.gpsimd.memset(ones, 1.0)
                            nc.gpsimd.memset(V[t][:, H * VG:], 0.0)
                    return [lambda t=t: step(t) for t in range(NT)]

                QK = {}

                def make_proj(ht, pools=None, plim=99):
                    QZ = [qkp.tile([128, NQ], BF16, name=f"QZ{j}",
                                   tag=f"QZ{j}", bufs=2) for j in range(2)]
                    # K^T in 4 column-chunk tiles so score step k only waits
                    # on chunk k//4 (faster startup, smoother pair starts)
                    KTc = [qkp.tile([128, 512], BF16, name=f"KTc{j}",
                                    tag=f"KTc{j}", bufs=2) for j in range(4)]
                    QK[ht] = (QZ, KTc)
                    nc.gpsimd.memset(QZ[0][DH:128, :], 0.0)
                    nc.gpsimd.memset(QZ[1][0:DH, :], 0.0)
                    steps = []
                    nstep = [0]
                    for (is_q, co, nn) in ((True, ht * 128, NQ),
                                           (False, D + ht * 128, N)):
                        wpB = projin.tile([128, DT * 128], BF16,
                                          name="wqkB", tag="wqkB", bufs=2)
                        wp = [wpB[:, d * 128:(d + 1) * 128]
                              for d in range(DT)]
                        nc.sync.dma_start(
                            wpB[:].rearrange("p (a n) -> p a n", n=128),
                            wqkv_d.ap()[:, co:co + 128].rearrange(
                                "(a p) n -> p a n", p=128))

                        def nb_step(is_q=is_q, wp=wp, nb=0):
                            if pools is None or nstep[0] >= plim:
                                nstep[0] += 1
                                ps = psA.tile([128, 512], F32, name="psA",
                                              tag="psA")
                            else:
                                pool, pname = pools[nstep[0] % len(pools)]
                                nstep[0] += 1
                                ps = pool.tile([128, 512], F32, name=pname,
                                               tag=pname)
                            for d in range(DT):
                                nc.tensor.matmul(
                                    ps[:],
                                    wp[d],
                                    xT_ap(d, nb * 512, (nb + 1) * 512),
                                    start=(d == 0), stop=(d == DT - 1))
                            if not is_q:
                                nc.vector.tensor_copy(KTc[nb][:], ps[:])
                            else:
                                nbs = slice(nb * 512, (nb + 1) * 512)
                                nc.vector.tensor_copy(
                                    QZ[0][0:DH, nbs], ps[0:DH, :])
                                nc.vector.tensor_copy(
                                    QZ[1][DH:128, nbs], ps[DH:128, :])
                        for nb in range(nn // 512):
                            steps.append(lambda f=nb_step, nb=nb: f(nb=nb))
                    return steps

                def pump(n):
                    for _ in range(n):
                        if v_steps:
                            v_steps.pop(0)()

                # ---- attn@V slot for prev half at tile k ----
                def av_step(h, PTl, po, k):
                    ht, hp = divmod(h, 2)
                    for qb in range(2):
                        nc.tensor.matmul(
                            po[qb][:],
                            V[k][:, h * VG:h * VG + 128],
                            PTl[k][:, qb * 512:(qb + 1) * 512],
                            start=(k == 0), stop=(k == NT - 1))

                DD = {}  # h -> [1, NQ] denominator row

                def av_drain(h, po):
                    ht, hp = divmod(h, 2)
                    DD[h] = smallp.tile([1, NQ], F32, name=f"DD{h % 2}",
                                        tag=f"DD{h % 2}")
                    for qb in range(2):
                        qs = slice(qb * 512, (qb + 1) * 512)
                        nc.vector.tensor_copy(
                            AOT[ht][hp * DH:(hp + 1) * DH, qs],
                            po[qb][0:DH, :])
                        nc.vector.tensor_copy(DD[h][0:1, qs],
                                              po[qb][VG - 1:VG, :])

                def normalize_pair(ht):
                    RB = []
                    for hp in range(2):
                        rf = smallp.tile([1, NQ], F32, name=f"Rf{hp}",
                                         tag=f"Rf{hp}", bufs=1)
                        rb = smallp.tile([1, NQ], BF16, name=f"Rb{hp}",
                                         tag=f"Rb{hp}", bufs=1)
                        nc.vector.reciprocal_approx_fast(rf[:],
                                                         DD[2 * ht + hp][:])
                        nc.vector.tensor_copy(rb[:], rf[:])
                        RB.append(rb)
                    for qb in range(2):
                        qs = slice(qb * 512, (qb + 1) * 512)
                        rbp = psA.tile([128, 512], F32, name="psA",
                                       tag="psA")
                        nc.tensor.matmul(rbp[:], E1[:], RB[0][:, qs],
                                         start=True, stop=False)
                        nc.tensor.matmul(rbp[:], E2[:], RB[1][:, qs],
                                         start=False, stop=True)
                        nc.vector.tensor_mul(
                            AOT[ht][:, qs], AOT[ht][:, qs], rbp[:])

                def emit_exp(psS, PTtile, k):
                    if k in DVE_K:
                        scr = scrp.tile([128, NQ], F32, name="scr",
                                        tag="scr")
                        nc.vector._custom_dve(EXP_BASE, out=scr[:],
                                              in0=psS[:],
                                              s0=0.125 / 2048.0, s1=0.5)
                        nc.vector._custom_dve(EXP_SQ8, out=PTtile[:],
                                              in0=scr[:])
                    else:
                        nc.scalar.activation(
                            PTtile[:], psS[:],
                            mybir.ActivationFunctionType.Exp, scale=0.125)

                # ================= half-phases =================
                with tc.tile_pool(name="psS", bufs=2, space="PSUM") as psSp:
                    steps0 = make_proj(0, pools=[(psA, "psA"),
                                                 (psSp, "psS")], plim=3)
                    for c in (1, 2, 3):
                        dma_xtc(c)
                    for step in steps0[:3]:
                        step()
                    v_steps.extend(steps0[3:])

                    prev = None  # (h, PTl, po) of the half in flight

                    for h in range(2 * NP):
                        ht, hp = divmod(h, 2)
                        if hp == 0:
                            if ht == 4:
                                load_tail_weights()
                            if ht >= 2:
                                normalize_pair(ht - 2)
                            if ht == 0:
                                v_steps.extend(make_v_panel(0))
                            if ht == 1:
                                v_steps.extend(make_v_panel(1))
                            if ht == 3:
                                v_steps.extend(make_v_panel(2))
                            if ht + 1 < NP:
                                v_steps.extend(make_proj(ht + 1))
                        QZ, KTc = QK[ht]

                        if prev is not None:
                            po = [pop.tile([128, 512], F32, name=f"po{qb}",
                                           tag=f"po{qb}") for qb in range(2)]
                            prev = (prev[0], prev[1], po)
                        PTl = [ptp.tile([128, NQ], BF16, name=f"PT{hp}_{k}",
                                        tag=f"PT{hp}_{k}")
                               for k in range(NT)]

                        for k in range(NT):
                            psS = psSp.tile([128, NQ], F32, name="psS",
                                            tag="psS")
                            for qb in range(2):
                                qs = slice(qb * 512, (qb + 1) * 512)
                                nc.tensor.matmul(
                                    psS[:, qs],
                                    KTc[k // 4][:, (k % 4) * 128:
                                                 (k % 4 + 1) * 128],
                                    QZ[hp][:, qs],
                                    start=True, stop=True)
                            emit_exp(psS, PTl[k], k)
                            if prev is not None:
                                av_step(prev[0], prev[1], prev[2], k)
                            pump(1)
                        if prev is not None:
                            av_drain(prev[0], prev[2])
                        prev = (h, PTl, None)

                        if debug_taps and h == 0:
                            nc.sync.dma_start(taps["tap_QT0"].ap(), QZ[0][:])
                            for j in range(4):
                                nc.sync.dma_start(
                                    taps["tap_KT0"].ap()[:, j * 512:
                                                         (j + 1) * 512],
                                    KTc[j][:])
                            nc.sync.dma_start(taps["tap_PTA4"].ap(),
                                              PTl[4][:])
                            nc.sync.dma_start(taps["tap_PTA5"].ap(),
                                              PTl[5][:])

                # ---- epilogue: last half's attn@V, normalizes, out-proj ----
                with tc.tile_pool(name="psC", bufs=4, space="PSUM") as psC:
                    h, PTl, _ = prev
                    po = [pop.tile([128, 512], F32, name=f"po{qb}",
                                   tag=f"po{qb}") for qb in range(2)]
                    for k in range(NT):
                        av_step(h, PTl, po, k)
                    av_drain(h, po)
                    normalize_pair(NP - 2)
                    normalize_pair(NP - 1)

                    groups = [(qt, fo, fsz) for qt in range(QT_TILES)
                              for (fo, fsz) in ((0, 512), (512, 256))]
                    for (qt, fo, fsz) in groups:
                        ps = psC.tile([128, 512], F32, name="psF", tag="psF")
                        for i in range(DT):
                            nc.tensor.matmul(
                                ps[:, :fsz],
                                AOT[i][:, qt * 128:(qt + 1) * 128],
                                WOB[:, i * D + fo:i * D + fo + fsz],
                                start=(i == 0), stop=(i == DT - 1))
                        ot = outsp.tile([128, 512], F32, name="ot", tag="ot")
                        nc.vector.tensor_add(
                            ot[:, :fsz], ps[:, :fsz], BIAS[:, fo:fo + fsz])
                        nc.sync.dma_start(
                            out_d.ap()[qt * 128:(qt + 1) * 128, fo:fo + fsz],
                            ot[:, :fsz])

    nc.compile()
    return nc


_NC = None


def _get_nc():
    global _NC
    if _NC is None:
        _NC = build()
    return _NC


def make_in_maps(x, w_qkv, w_out, b_out):
    import ml_dtypes
    x = np.asarray(x, np.float32)
    w_qkv = np.ascontiguousarray(np.asarray(w_qkv, ml_dtypes.bfloat16))
    w_out = np.ascontiguousarray(np.asarray(w_out, ml_dtypes.bfloat16))
    bias = np.ascontiguousarray(
        np.broadcast_to(np.asarray(b_out, np.float32)[None, :], (128, D)))
    in_maps = []
    for c in range(N_CORES):
        b, half = divmod(c, 2)
        xb = x[b]
        qoff = half * NQ
        # query half first; key order permutation is harmless
        xperm = np.vstack([xb[qoff:qoff + NQ], xb[NQ - qoff:2 * NQ - qoff]])
        in_maps.append({
            "xT": np.ascontiguousarray(xperm.T.astype(ml_dtypes.bfloat16)),
            "wqkv": w_qkv,
            "wout": w_out,
            "bias": bias,
        })
    return in_maps


def run(in_maps, trace=False, **kw):
    return run_bass_kernel_spmd(_get_nc(), in_maps,
                                core_ids=list(range(N_CORES)),
                                trace=trace, **kw)


def assemble(results):
    out = np.empty((B, N, D), np.float32)
    for c in range(N_CORES):
        b, half = divmod(c, 2)
        out[b, half * NQ:(half + 1) * NQ, :] = results[c]["out"]
    return out


def kernel(x, w_qkv, w_out, b_out):
    res = run(make_in_maps(x, w_qkv, w_out, b_out))
    return assemble(res.results)


# revision 63
# speedup vs baseline: 1.0090x; 1.0090x over previous
"""Multi-head attention (B=4, N=2048, D=768, H=12, Dh=64) on 8 TRN2 NeuronCores.

Sharding: core c -> batch b = c//2, query rows half = c%2 (1024 rows each).
Each core computes all 12 heads for its (batch, query-half) against the full
2048-key sequence, so outputs are disjoint and no collective is needed.

Head-sequential eager pipeline: the kernel is a continuous stream of
"half-phases" (one per head). In half h, step k emits:
  - scores(h, k): two row-tiled K=64 matmuls (the head's 64 q/k dims live in
    array rows hp*64) -> S^T[128 keys, 1024 q] in a psS generation,
  - exp(h, k) on ACT (or, for k in DVE_K, a 2-instruction custom DVE op
    computing (1 + t + t^2/2)^2048 by repeated squaring),
  - attn@V of the PREVIOUS half at tile k: two [K=128, M=128, N=512] matmuls
    against the V panel (65 cols per head: 64 dims + ones column whose output
    row 64 is the softmax denominator),
  - one projection/V-panel filler step (next pair's Q^T/K^T, V panels).
The exp stream never waits on a serial prologue; the scalar engine runs
near-continuously.  Denominator rows are normalized baseline-style: fast
reciprocal + two K<=1 selector matmuls broadcast 1/d to the head's partition
rows, then one multiply.
"""

import numpy as np

import concourse.bass as bass
import concourse.bacc as bacc
import concourse.mybir as mybir
import concourse.tile as tile
from concourse.bass_utils import run_bass_kernel_spmd

N_CORES = 8
B, N, D = 4, 2048, 768
H, DH = 12, 64
NQ = 1024           # query rows per core
COLS = 3 * D        # 2304 qkv columns
DT = D // 128       # 6 partition tiles of the model dim
NT = N // 128       # 16 key tiles
QT_TILES = NQ // 128
NP = DT             # 6 head pairs
VG = DH + 1         # 65: head group width in V (64 cols + ones)

F32 = mybir.dt.float32
BF16 = mybir.dt.bfloat16

# k-tiles whose exp runs on the DVE via the custom squaring ops
DVE_K = (5, 11)


# ---------------- custom DVE exp ----------------
# exp(0.125*s) = u^2048, u = 1 + t + t^2/2 = ((t+1)^2 + 1)/2, t = 0.125*s/2048
def _register_exp_ops():
    import concourse.dve_ops as dve_ops
    from concourse.dve_spec import Spec, Src0, C0, C1, One, sq, lower
    from concourse.dve_uop import DveOpSpec
    from concourse.dve_table_gen import dve_ver_for

    def by_name(name):
        for o in dve_ops.OPS:
            if o.name == name:
                return o
        return None

    got = (by_name("EXP2K_BASE_ANT"), by_name("EXP2K_SQ8_ANT"))
    if got[0] is not None:
        return got

    a = sq(Src0 * C0 + One) + One
    body1 = sq(sq(sq(a * C1)))

    def ref1(in0, in1, s0, s1, imm2):
        u = ((in0.astype(np.float64) * s0 + 1.0) ** 2 + 1.0) * s1
        return (u ** 8).astype(np.float32)

    body2 = sq(sq(sq(sq(sq(sq(sq(sq(Src0))))))))

    def ref2(in0, in1, s0, s1, imm2):
        return (in0.astype(np.float64) ** 256).astype(np.float32)

    ver = dve_ver_for("TRN2")
    ops = []
    for name, body, ref in (("EXP2K_BASE_ANT", body1, ref1),
                            ("EXP2K_SQ8_ANT", body2, ref2)):
        spec = Spec(body=body, reference=ref)
        row = max(dve_ops._SUB_OPCODE_FOR_NAME.values()) + 1
        assert row < 0x20
        tmp = DveOpSpec(name=name, opcode=row, uops=lower(spec, ver=ver),
                        rd1_en=False)
        op = dve_ops.DveOp(name, spec, subdim=False,
                           uops_sha={ver: tmp.sha(ver)})
        dve_ops._SUB_OPCODE_FOR_NAME[name] = row
        dve_ops.OPS.append(op)
        dve_ops.CUSTOM_DVE_SPECS[name] = spec
        ops.append(op)
    return tuple(ops)


EXP_BASE, EXP_SQ8 = _register_exp_ops()


def build(debug_taps=False):
    nc = bacc.Bacc("TRN2", target_bir_lowering=False, debug=False,
                   num_devices=N_CORES)

    xT_d = nc.dram_tensor("xT", [D, N], BF16, kind="ExternalInput")
    wqkv_d = nc.dram_tensor("wqkv", [D, COLS], BF16, kind="ExternalInput")
    wout_d = nc.dram_tensor("wout", [D, D], BF16, kind="ExternalInput")
    bias_d = nc.dram_tensor("bias", [128, D], F32, kind="ExternalInput")
    out_d = nc.dram_tensor("out", [NQ, D], F32, kind="ExternalOutput")

    taps = {}
    if debug_taps:
        for name, shape, dt in (("tap_QT0", [128, NQ], BF16),
                                ("tap_KT0", [128, N], BF16),
                                ("tap_PTA4", [128, NQ], BF16),
                                ("tap_PTA5", [128, NQ], BF16)):
            taps[name] = nc.dram_tensor(name, shape, dt,
                                        kind="ExternalOutput")

    with tile.TileContext(nc) as tc:
        with tc.tile_pool(name="persist", bufs=1) as pp, \
             tc.tile_pool(name="small", bufs=2) as smallp, \
             tc.tile_pool(name="outs", bufs=3) as outsp:

            # V with a ones column per head and 63 pad cols so the attn@V
            # stationary can be a full 128-col slice
            V = [pp.tile([128, H * VG + 63], BF16, name=f"V{i}", tag=f"V{i}")
                 for i in range(NT)]
            AOT = [pp.tile([128, NQ], BF16, name=f"AOT{i}", tag=f"AOT{i}")
                   for i in range(NP)]
            WOB = pp.tile([128, DT * D], BF16, name="WOB", tag="WOB")
            BIAS = pp.tile([128, D], F32, name="BIAS", tag="BIAS")
            E1 = pp.tile([1, 128], BF16, name="E1", tag="E1")
            E2 = pp.tile([1, 128], BF16, name="E2", tag="E2")

            nc.gpsimd.memset(E1[:], 0.0)
            nc.gpsimd.memset(E2[:], 0.0)
            nc.gpsimd.memset(E1[0:1, 0:DH], 1.0)
            nc.gpsimd.memset(E2[0:1, DH:128], 1.0)

            def load_tail_weights():
                nc.sync.dma_start(BIAS[:], bias_d.ap())
                nc.sync.dma_start(
                    WOB[:].rearrange("p (a n) -> p a n", n=D),
                    wout_d.ap().rearrange("(a p) n -> p a n", p=128))

            with tc.tile_pool(name="projin", bufs=1) as projin, \
                 tc.tile_pool(name="qk", bufs=1) as qkp, \
                 tc.tile_pool(name="pt", bufs=1) as ptp, \
                 tc.tile_pool(name="scr", bufs=1) as scrp, \
                 tc.tile_pool(name="psA", bufs=2, space="PSUM") as psA, \
                 tc.tile_pool(name="po", bufs=1, space="PSUM") as pop:

                xTC = [projin.tile([128, DT * 512], BF16, name=f"xTC{c}",
                                   tag=f"xTC{c}") for c in range(4)]

                def dma_xtc(c):
                    nc.sync.dma_start(
                        xTC[c][:].rearrange("p (a n) -> p a n", n=512),
                        xT_d.ap()[:, c * 512:(c + 1) * 512].rearrange(
                            "(a p) n -> p a n", p=128))

                dma_xtc(0)

                def xT_ap(d, lo, hi):
                    c, off = divmod(lo, 512)
                    assert (hi - 1) // 512 == c
                    return xTC[c][:, d * 512 + off:d * 512 + off + (hi - lo)]

                # ---- filler steps (V panels + next-pair projections) ----
                v_steps = []

                def make_v_panel(vp):
                    co = 2 * D + vp * 256
                    wvB = projin.tile([128, DT * 256], BF16, name="wvB",
                                      tag="wvB", bufs=1)
                    wv = [wvB[:, d * 256:(d + 1) * 256] for d in range(DT)]
                    nc.sync.dma_start(
                        wvB[:].rearrange("p (a n) -> p a n", n=256),
                        wqkv_d.ap()[:, co:co + 256].rearrange(
                            "(a p) n -> p a n", p=128))

                    def step(t):
                        ps = psA.tile([128, 512], F32, name="psA", tag="psA")
                        for d in range(DT):
                            nc.tensor.matmul(
                                ps[:, :256],
                                xT_ap(d, t * 128, (t + 1) * 128),
                                wv[d],
                                start=(d == 0), stop=(d == DT - 1))
                        dst = V[t][:, 0:H * VG].rearrange(
                            "p (h c) -> p h c", c=VG)
                        nc.vector.tensor_copy(
                            dst[:, vp * 4:(vp + 1) * 4, 0:DH],
                            ps[:, :256].rearrange("p (h c) -> p h c", c=DH))
                        if vp == 0:
                            ones = V[t][:, 0:H * VG].rearrange(
                                "p (h c) -> p h c", c=VG)[:, :, DH:VG]
                            nc.gpsimd.memset(ones, 1.0)
                            nc.gpsimd.memset(V[t][:, H * VG:], 0.0)
                    return [lambda t=t: step(t) for t in range(NT)]

                QK = {}

                def make_proj(ht, pools=None, plim=99):
                    QZ = [qkp.tile([128, NQ], BF16, name=f"QZ{j}",
                                   tag=f"QZ{j}", bufs=2) for j in range(2)]
                    # K^T in 4 column-chunk tiles so score step k only waits
                    # on chunk k//4 (faster startup, smoother pair starts)
                    KTc = [qkp.tile([128, 512], BF16, name=f"KTc{j}",
                                    tag=f"KTc{j}", bufs=2) for j in range(4)]
                    QK[ht] = (QZ, KTc)
                    nc.gpsimd.memset(QZ[0][DH:128, :], 0.0)
                    nc.gpsimd.memset(QZ[1][0:DH, :], 0.0)
                    steps = []
                    nstep = [0]
                    for (is_q, co, nn) in ((True, ht * 128, NQ),
                                           (False, D + ht * 128, N)):
                        wpB = projin.tile([128, DT * 128], BF16,
                                          name="wqkB", tag="wqkB", bufs=2)
                        wp = [wpB[:, d * 128:(d + 1) * 128]
                              for d in range(DT)]
                        nc.sync.dma_start(
                            wpB[:].rearrange("p (a n) -> p a n", n=128),
                            wqkv_d.ap()[:, co:co + 128].rearrange(
                                "(a p) n -> p a n", p=128))

                        def nb_step(is_q=is_q, wp=wp, nb=0):
                            if pools is None or nstep[0] >= plim:
                                nstep[0] += 1
                                ps = psA.tile([128, 512], F32, name="psA",
                                              tag="psA")
                            else:
                                pool, pname = pools[nstep[0] % len(pools)]
                                nstep[0] += 1
                                ps = pool.tile([128, 512], F32, name=pname,
                                               tag=pname)
                            for d in range(DT):
                                nc.tensor.matmul(
                                    ps[:],
                                    wp[d],
                                    xT_ap(d, nb * 512, (nb + 1) * 512),
                                    start=(d == 0), stop=(d == DT - 1))
                            if not is_q:
                                nc.vector.tensor_copy(KTc[nb][:], ps[:])
                            else:
                                nbs = slice(nb * 512, (nb + 1) * 512)
                                nc.vector.tensor_copy(
                                    QZ[0][0:DH, nbs], ps[0:DH, :])
                                nc.vector.tensor_copy(
                                    QZ[1][DH:128, nbs], ps[DH:128, :])
                        for nb in range(nn // 512):
                            steps.append(lambda f=nb_step, nb=nb: f(nb=nb))
                    return steps

                def pump(n):
                    for _ in range(n):
                        if v_steps:
                            v_steps.pop(0)()

                # ---- attn@V slot for prev half at tile k ----
                def av_step(h, PTl, po, k):
                    ht, hp = divmod(h, 2)
                    for qb in range(2):
                        nc.tensor.matmul(
                            po[qb][:],
                            V[k][:, h * VG:h * VG + 128],
                            PTl[k][:, qb * 512:(qb + 1) * 512],
                            start=(k == 0), stop=(k == NT - 1))

                DD = {}  # h -> [1, NQ] denominator row

                def av_drain(h, po):
                    ht, hp = divmod(h, 2)
                    DD[h] = smallp.tile([1, NQ], F32, name=f"DD{h % 2}",
                                        tag=f"DD{h % 2}")
                    for qb in range(2):
                        qs = slice(qb * 512, (qb + 1) * 512)
                        nc.vector.tensor_copy(
                            AOT[ht][hp * DH:(hp + 1) * DH, qs],
                            po[qb][0:DH, :])
                        nc.vector.tensor_copy(DD[h][0:1, qs],
                                              po[qb][VG - 1:VG, :])

                def normalize_pair(ht):
                    RB = []
                    for hp in range(2):
                        rf = smallp.tile([1, NQ], F32, name=f"Rf{hp}",
                                         tag=f"Rf{hp}", bufs=1)
                        rb = smallp.tile([1, NQ], BF16, name=f"Rb{hp}",
                                         tag=f"Rb{hp}", bufs=1)
                        nc.vector.reciprocal_approx_fast(rf[:],
                                                         DD[2 * ht + hp][:])
                        nc.vector.tensor_copy(rb[:], rf[:])
                        RB.append(rb)
                    for qb in range(2):
                        qs = slice(qb * 512, (qb + 1) * 512)
                        rbp = psA.tile([128, 512], F32, name="psA",
                                       tag="psA")
                        nc.tensor.matmul(rbp[:], E1[:], RB[0][:, qs],
                                         start=True, stop=False)
                        nc.tensor.matmul(rbp[:], E2[:], RB[1][:, qs],
                                         start=False, stop=True)
                        nc.vector.tensor_mul(
                            AOT[ht][:, qs], AOT[ht][:, qs], rbp[:])

                def emit_exp(psS, PTtile, k):
                    if k in DVE_K:
                        scr = scrp.tile([128, NQ], F32, name="scr",
                                        tag="scr")
                        nc.vector._custom_dve(EXP_BASE, out=scr[:],
                                              in0=psS[:],
                                              s0=0.125 / 2048.0, s1=0.5)
                        nc.vector._custom_dve(EXP_SQ8, out=PTtile[:],
                                              in0=scr[:])
                    else:
                        nc.scalar.activation(
                            PTtile[:], psS[:],
                            mybir.ActivationFunctionType.Exp, scale=0.125)

                # ================= half-phases =================
                with tc.tile_pool(name="psS", bufs=2, space="PSUM") as psSp:
                    steps0 = make_proj(0, pools=[(psA, "psA"),
                                                 (psSp, "psS")], plim=3)
                    for c in (1, 2, 3):
                        dma_xtc(c)
                    for step in steps0[:3]:
                        step()
                    v_steps.extend(steps0[3:])

                    prev = None  # (h, PTl, po) of the half in flight

                    for h in range(2 * NP):
                        ht, hp = divmod(h, 2)
                        if hp == 0:
                            if ht == 4:
                                load_tail_weights()
                            if ht == 0:
                                v_steps.extend(make_v_panel(0))
                            if ht == 1:
                                v_steps.extend(make_v_panel(1))
                            if ht == 3:
                                v_steps.extend(make_v_panel(2))
                            if ht + 1 < NP:
                                v_steps.extend(make_proj(ht + 1))
                        QZ, KTc = QK[ht]

                        if prev is not None:
                            po = [pop.tile([128, 512], F32, name=f"po{qb}",
                                           tag=f"po{qb}") for qb in range(2)]
                            prev = (prev[0], prev[1], po)
                        PTl = [ptp.tile([128, NQ], BF16, name=f"PT{hp}_{k}",
                                        tag=f"PT{hp}_{k}")
                               for k in range(NT)]

                        for k in range(NT):
                            psS = psSp.tile([128, NQ], F32, name="psS",
                                            tag="psS")
                            for qb in range(2):
                                qs = slice(qb * 512, (qb + 1) * 512)
                                nc.tensor.matmul(
                                    psS[:, qs],
                                    KTc[k // 4][:, (k % 4) * 128:
                                                 (k % 4 + 1) * 128],
                                    QZ[hp][:, qs],
                                    start=True, stop=True)
                            emit_exp(psS, PTl[k], k)
                            if prev is not None:
                                av_step(prev[0], prev[1], prev[2], k)
                            if hp == 0 and k == 3 and ht >= 2:
                                normalize_pair(ht - 2)
                            pump(1)
                        if prev is not None:
                            av_drain(prev[0], prev[2])
                        prev = (h, PTl, None)

                        if debug_taps and h == 0:
                            nc.sync.dma_start(taps["tap_QT0"].ap(), QZ[0][:])
                            for j in range(4):
                                nc.sync.dma_start(
                                    taps["tap_KT0"].ap()[:, j * 512:
                                                         (j + 1) * 512],
                                    KTc[j][:])
                            nc.sync.dma_start(taps["tap_PTA4"].ap(),
                                              PTl[4][:])
                            nc.sync.dma_start(taps["tap_PTA5"].ap(),
                                              PTl[5][:])

                # ---- epilogue: last half's attn@V, normalizes, out-proj ----
                with tc.tile_pool(name="psC", bufs=4, space="PSUM") as psC:
                    h, PTl, _ = prev
                    po = [pop.tile([128, 512], F32, name=f"po{qb}",
                                   tag=f"po{qb}") for qb in range(2)]
                    for k in range(NT):
                        av_step(h, PTl, po, k)
                    av_drain(h, po)
                    normalize_pair(NP - 2)
                    normalize_pair(NP - 1)

                    groups = [(qt, fo, fsz) for qt in range(QT_TILES)
                              for (fo, fsz) in ((0, 512), (512, 256))]
                    for (qt, fo, fsz) in groups:
                        ps = psC.tile([128, 512], F32, name="psF", tag="psF")
                        for i in range(DT):
                            nc.tensor.matmul(
                                ps[:, :fsz],
                                AOT[i][:, qt * 128:(qt + 1) * 128],
                                WOB[:, i * D + fo:i * D + fo + fsz],
                                start=(i == 0), stop=(i == DT - 1))
                        ot = outsp.tile([128, 512], F32, name="ot", tag="ot")
                        nc.vector.tensor_add(
                            ot[:, :fsz], ps[:, :fsz], BIAS[:, fo:fo + fsz])
                        nc.sync.dma_start(
                            out_d.ap()[qt * 128:(qt + 1) * 128, fo:fo + fsz],
                            ot[:, :fsz])

    nc.compile()
    return nc


_NC = None


def _get_nc():
    global _NC
    if _NC is None:
        _NC = build()
    return _NC


def make_in_maps(x, w_qkv, w_out, b_out):
    import ml_dtypes
    x = np.asarray(x, np.float32)
    w_qkv = np.ascontiguousarray(np.asarray(w_qkv, ml_dtypes.bfloat16))
    w_out = np.ascontiguousarray(np.asarray(w_out, ml_dtypes.bfloat16))
    bias = np.ascontiguousarray(
        np.broadcast_to(np.asarray(b_out, np.float32)[None, :], (128, D)))
    in_maps = []
    for c in range(N_CORES):
        b, half = divmod(c, 2)
        xb = x[b]
        qoff = half * NQ
        # query half first; key order permutation is harmless
        xperm = np.vstack([xb[qoff:qoff + NQ], xb[NQ - qoff:2 * NQ - qoff]])
        in_maps.append({
            "xT": np.ascontiguousarray(xperm.T.astype(ml_dtypes.bfloat16)),
            "wqkv": w_qkv,
            "wout": w_out,
            "bias": bias,
        })
    return in_maps


def run(in_maps, trace=False, **kw):
    return run_bass_kernel_spmd(_get_nc(), in_maps,
                                core_ids=list(range(N_CORES)),
                                trace=trace, **kw)


def assemble(results):
    out = np.empty((B, N, D), np.float32)
    for c in range(N_CORES):
        b, half = divmod(c, 2)
        out[b, half * NQ:(half + 1) * NQ, :] = results[c]["out"]
    return out


def kernel(x, w_qkv, w_out, b_out):
    res = run(make_in_maps(x, w_qkv, w_out, b_out))
    return assemble(res.results)


# revision 64
# speedup vs baseline: 1.0130x; 1.0039x over previous
"""Multi-head attention (B=4, N=2048, D=768, H=12, Dh=64) on 8 TRN2 NeuronCores.

Sharding: core c -> batch b = c//2, query rows half = c%2 (1024 rows each).
Each core computes all 12 heads for its (batch, query-half) against the full
2048-key sequence, so outputs are disjoint and no collective is needed.

Head-sequential eager pipeline: the kernel is a continuous stream of
"half-phases" (one per head). In half h, step k emits:
  - scores(h, k): two row-tiled K=64 matmuls (the head's 64 q/k dims live in
    array rows hp*64) -> S^T[128 keys, 1024 q] in a psS generation,
  - exp(h, k) on ACT (or, for k in DVE_K, a 2-instruction custom DVE op
    computing (1 + t + t^2/2)^2048 by repeated squaring),
  - attn@V of the PREVIOUS half at tile k: two [K=128, M=128, N=512] matmuls
    against the V panel (65 cols per head: 64 dims + ones column whose output
    row 64 is the softmax denominator),
  - one projection/V-panel filler step (next pair's Q^T/K^T, V panels).
The exp stream never waits on a serial prologue; the scalar engine runs
near-continuously.  Denominator rows are normalized baseline-style: fast
reciprocal + two K<=1 selector matmuls broadcast 1/d to the head's partition
rows, then one multiply.
"""

import numpy as np

import concourse.bass as bass
import concourse.bacc as bacc
import concourse.mybir as mybir
import concourse.tile as tile
from concourse.bass_utils import run_bass_kernel_spmd

N_CORES = 8
B, N, D = 4, 2048, 768
H, DH = 12, 64
NQ = 1024           # query rows per core
COLS = 3 * D        # 2304 qkv columns
DT = D // 128       # 6 partition tiles of the model dim
NT = N // 128       # 16 key tiles
QT_TILES = NQ // 128
NP = DT             # 6 head pairs
VG = DH + 1         # 65: head group width in V (64 cols + ones)

F32 = mybir.dt.float32
BF16 = mybir.dt.bfloat16

# k-tiles whose exp runs on the DVE via the custom squaring ops
DVE_K = (3, 9)


# ---------------- custom DVE exp ----------------
# exp(0.125*s) = u^2048, u = 1 + t + t^2/2 = ((t+1)^2 + 1)/2, t = 0.125*s/2048
def _register_exp_ops():
    import concourse.dve_ops as dve_ops
    from concourse.dve_spec import Spec, Src0, C0, C1, One, sq, lower
    from concourse.dve_uop import DveOpSpec
    from concourse.dve_table_gen import dve_ver_for

    def by_name(name):
        for o in dve_ops.OPS:
            if o.name == name:
                return o
        return None

    got = (by_name("EXP2K_BASE_ANT"), by_name("EXP2K_SQ8_ANT"))
    if got[0] is not None:
        return got

    a = sq(Src0 * C0 + One) + One
    body1 = sq(sq(sq(a * C1)))

    def ref1(in0, in1, s0, s1, imm2):
        u = ((in0.astype(np.float64) * s0 + 1.0) ** 2 + 1.0) * s1
        return (u ** 8).astype(np.float32)

    body2 = sq(sq(sq(sq(sq(sq(sq(sq(Src0))))))))

    def ref2(in0, in1, s0, s1, imm2):
        return (in0.astype(np.float64) ** 256).astype(np.float32)

    ver = dve_ver_for("TRN2")
    ops = []
    for name, body, ref in (("EXP2K_BASE_ANT", body1, ref1),
                            ("EXP2K_SQ8_ANT", body2, ref2)):
        spec = Spec(body=body, reference=ref)
        row = max(dve_ops._SUB_OPCODE_FOR_NAME.values()) + 1
        assert row < 0x20
        tmp = DveOpSpec(name=name, opcode=row, uops=lower(spec, ver=ver),
                        rd1_en=False)
        op = dve_ops.DveOp(name, spec, subdim=False,
                           uops_sha={ver: tmp.sha(ver)})
        dve_ops._SUB_OPCODE_FOR_NAME[name] = row
        dve_ops.OPS.append(op)
        dve_ops.CUSTOM_DVE_SPECS[name] = spec
        ops.append(op)
    return tuple(ops)


EXP_BASE, EXP_SQ8 = _register_exp_ops()


def build(debug_taps=False):
    nc = bacc.Bacc("TRN2", target_bir_lowering=False, debug=False,
                   num_devices=N_CORES)

    xT_d = nc.dram_tensor("xT", [D, N], BF16, kind="ExternalInput")
    wqkv_d = nc.dram_tensor("wqkv", [D, COLS], BF16, kind="ExternalInput")
    wout_d = nc.dram_tensor("wout", [D, D], BF16, kind="ExternalInput")
    bias_d = nc.dram_tensor("bias", [128, D], F32, kind="ExternalInput")
    out_d = nc.dram_tensor("out", [NQ, D], F32, kind="ExternalOutput")

    taps = {}
    if debug_taps:
        for name, shape, dt in (("tap_QT0", [128, NQ], BF16),
                                ("tap_KT0", [128, N], BF16),
                                ("tap_PTA4", [128, NQ], BF16),
                                ("tap_PTA5", [128, NQ], BF16)):
            taps[name] = nc.dram_tensor(name, shape, dt,
                                        kind="ExternalOutput")

    with tile.TileContext(nc) as tc:
        with tc.tile_pool(name="persist", bufs=1) as pp, \
             tc.tile_pool(name="small", bufs=2) as smallp, \
             tc.tile_pool(name="outs", bufs=5) as outsp:

            # V with a ones column per head and 63 pad cols so the attn@V
            # stationary can be a full 128-col slice
            V = [pp.tile([128, H * VG + 63], BF16, name=f"V{i}", tag=f"V{i}")
                 for i in range(NT)]
            AOT = [pp.tile([128, NQ], BF16, name=f"AOT{i}", tag=f"AOT{i}")
                   for i in range(NP)]
            WOB = pp.tile([128, DT * D], BF16, name="WOB", tag="WOB")
            BIAS = pp.tile([128, D], F32, name="BIAS", tag="BIAS")
            E1 = pp.tile([1, 128], BF16, name="E1", tag="E1")
            E2 = pp.tile([1, 128], BF16, name="E2", tag="E2")

            nc.gpsimd.memset(E1[:], 0.0)
            nc.gpsimd.memset(E2[:], 0.0)
            nc.gpsimd.memset(E1[0:1, 0:DH], 1.0)
            nc.gpsimd.memset(E2[0:1, DH:128], 1.0)

            def load_tail_weights():
                nc.sync.dma_start(BIAS[:], bias_d.ap())
                nc.sync.dma_start(
                    WOB[:].rearrange("p (a n) -> p a n", n=D),
                    wout_d.ap().rearrange("(a p) n -> p a n", p=128))

            with tc.tile_pool(name="projin", bufs=1) as projin, \
                 tc.tile_pool(name="qk", bufs=1) as qkp, \
                 tc.tile_pool(name="pt", bufs=1) as ptp, \
                 tc.tile_pool(name="scr", bufs=1) as scrp, \
                 tc.tile_pool(name="psA", bufs=2, space="PSUM") as psA, \
                 tc.tile_pool(name="po", bufs=1, space="PSUM") as pop:

                xTC = [projin.tile([128, DT * 512], BF16, name=f"xTC{c}",
                                   tag=f"xTC{c}") for c in range(4)]

                def dma_xtc(c):
                    nc.sync.dma_start(
                        xTC[c][:].rearrange("p (a n) -> p a n", n=512),
                        xT_d.ap()[:, c * 512:(c + 1) * 512].rearrange(
                            "(a p) n -> p a n", p=128))

                dma_xtc(0)

                def xT_ap(d, lo, hi):
                    c, off = divmod(lo, 512)
                    assert (hi - 1) // 512 == c
                    return xTC[c][:, d * 512 + off:d * 512 + off + (hi - lo)]

                # ---- filler steps (V panels + next-pair projections) ----
                v_steps = []

                def make_v_panel(vp):
                    co = 2 * D + vp * 256
                    wvB = projin.tile([128, DT * 256], BF16, name="wvB",
                                      tag="wvB", bufs=1)
                    wv = [wvB[:, d * 256:(d + 1) * 256] for d in range(DT)]
                    nc.sync.dma_start(
                        wvB[:].rearrange("p (a n) -> p a n", n=256),
                        wqkv_d.ap()[:, co:co + 256].rearrange(
                            "(a p) n -> p a n", p=128))

                    def step(t):
                        ps = psA.tile([128, 512], F32, name="psA", tag="psA")
                        for d in range(DT):
                            nc.tensor.matmul(
                                ps[:, :256],
                                xT_ap(d, t * 128, (t + 1) * 128),
                                wv[d],
                                start=(d == 0), stop=(d == DT - 1))
                        dst = V[t][:, 0:H * VG].rearrange(
                            "p (h c) -> p h c", c=VG)
                        nc.vector.tensor_copy(
                            dst[:, vp * 4:(vp + 1) * 4, 0:DH],
                            ps[:, :256].rearrange("p (h c) -> p h c", c=DH))
                        if vp == 0:
                            ones = V[t][:, 0:H * VG].rearrange(
                                "p (h c) -> p h c", c=VG)[:, :, DH:VG]
                            nc.gpsimd.memset(ones, 1.0)
                            nc.gpsimd.memset(V[t][:, H * VG:], 0.0)
                    return [lambda t=t: step(t) for t in range(NT)]

                QK = {}

                def make_proj(ht, pools=None, plim=99):
                    QZ = [qkp.tile([128, NQ], BF16, name=f"QZ{j}",
                                   tag=f"QZ{j}", bufs=2) for j in range(2)]
                    # K^T in 4 column-chunk tiles so score step k only waits
                    # on chunk k//4 (faster startup, smoother pair starts)
                    KTc = [qkp.tile([128, 512], BF16, name=f"KTc{j}",
                                    tag=f"KTc{j}", bufs=2) for j in range(4)]
                    QK[ht] = (QZ, KTc)
                    nc.gpsimd.memset(QZ[0][DH:128, :], 0.0)
                    nc.gpsimd.memset(QZ[1][0:DH, :], 0.0)
                    steps = []
                    nstep = [0]
                    for (is_q, co, nn) in ((True, ht * 128, NQ),
                                           (False, D + ht * 128, N)):
                        wpB = projin.tile([128, DT * 128], BF16,
                                          name="wqkB", tag="wqkB", bufs=2)
                        wp = [wpB[:, d * 128:(d + 1) * 128]
                              for d in range(DT)]
                        nc.sync.dma_start(
                            wpB[:].rearrange("p (a n) -> p a n", n=128),
                            wqkv_d.ap()[:, co:co + 128].rearrange(
                                "(a p) n -> p a n", p=128))

                        def nb_step(is_q=is_q, wp=wp, nb=0):
                            if pools is None or nstep[0] >= plim:
                                nstep[0] += 1
                                ps = psA.tile([128, 512], F32, name="psA",
                                              tag="psA")
                            else:
                                pool, pname = pools[nstep[0] % len(pools)]
                                nstep[0] += 1
                                ps = pool.tile([128, 512], F32, name=pname,
                                               tag=pname)
                            for d in range(DT):
                                nc.tensor.matmul(
                                    ps[:],
                                    wp[d],
                                    xT_ap(d, nb * 512, (nb + 1) * 512),
                                    start=(d == 0), stop=(d == DT - 1))
                            if not is_q:
                                nc.vector.tensor_copy(KTc[nb][:], ps[:])
                            else:
                                nbs = slice(nb * 512, (nb + 1) * 512)
                                nc.vector.tensor_copy(
                                    QZ[0][0:DH, nbs], ps[0:DH, :])
                                nc.vector.tensor_copy(
                                    QZ[1][DH:128, nbs], ps[DH:128, :])
                        for nb in range(nn // 512):
                            steps.append(lambda f=nb_step, nb=nb: f(nb=nb))
                    return steps

                def pump(n):
                    for _ in range(n):
                        if v_steps:
                            v_steps.pop(0)()

                # ---- attn@V slot for prev half at tile k ----
                def av_step(h, PTl, po, k):
                    ht, hp = divmod(h, 2)
                    for qb in range(2):
                        nc.tensor.matmul(
                            po[qb][:],
                            V[k][:, h * VG:h * VG + 128],
                            PTl[k][:, qb * 512:(qb + 1) * 512],
                            start=(k == 0), stop=(k == NT - 1))

                DD = {}  # h -> [1, NQ] denominator row

                def av_drain(h, po):
                    ht, hp = divmod(h, 2)
                    DD[h] = smallp.tile([1, NQ], F32, name=f"DD{h % 2}",
                                        tag=f"DD{h % 2}")
                    for qb in range(2):
                        qs = slice(qb * 512, (qb + 1) * 512)
                        nc.vector.tensor_copy(
                            AOT[ht][hp * DH:(hp + 1) * DH, qs],
                            po[qb][0:DH, :])
                        nc.vector.tensor_copy(DD[h][0:1, qs],
                                              po[qb][VG - 1:VG, :])

                def normalize_pair(ht):
                    RB = []
                    for hp in range(2):
                        rf = smallp.tile([1, NQ], F32, name=f"Rf{hp}",
                                         tag=f"Rf{hp}", bufs=1)
                        rb = smallp.tile([1, NQ], BF16, name=f"Rb{hp}",
                                         tag=f"Rb{hp}", bufs=1)
                        nc.vector.reciprocal_approx_fast(rf[:],
                                                         DD[2 * ht + hp][:])
                        nc.vector.tensor_copy(rb[:], rf[:])
                        RB.append(rb)
                    for qb in range(2):
                        qs = slice(qb * 512, (qb + 1) * 512)
                        rbp = psA.tile([128, 512], F32, name="psA",
                                       tag="psA")
                        nc.tensor.matmul(rbp[:], E1[:], RB[0][:, qs],
                                         start=True, stop=False)
                        nc.tensor.matmul(rbp[:], E2[:], RB[1][:, qs],
                                         start=False, stop=True)
                        nc.vector.tensor_mul(
                            AOT[ht][:, qs], AOT[ht][:, qs], rbp[:])

                def emit_exp(psS, PTtile, k):
                    if k in DVE_K:
                        scr = scrp.tile([128, NQ], F32, name="scr",
                                        tag="scr")
                        nc.vector._custom_dve(EXP_BASE, out=scr[:],
                                              in0=psS[:],
                                              s0=0.125 / 2048.0, s1=0.5)
                        nc.vector._custom_dve(EXP_SQ8, out=PTtile[:],
                                              in0=scr[:])
                    else:
                        nc.scalar.activation(
                            PTtile[:], psS[:],
                            mybir.ActivationFunctionType.Exp, scale=0.125)

                # ================= half-phases =================
                with tc.tile_pool(name="psS", bufs=2, space="PSUM") as psSp:
                    steps0 = make_proj(0, pools=[(psA, "psA"),
                                                 (psSp, "psS")], plim=3)
                    for c in (1, 2, 3):
                        dma_xtc(c)
                    for step in steps0[:3]:
                        step()
                    v_steps.extend(steps0[3:])

                    prev = None  # (h, PTl, po) of the half in flight

                    for h in range(2 * NP):
                        ht, hp = divmod(h, 2)
                        if hp == 0:
                            if ht == 4:
                                load_tail_weights()
                            if ht == 0:
                                v_steps.extend(make_v_panel(0))
                            if ht == 1:
                                v_steps.extend(make_v_panel(1))
                            if ht == 3:
                                v_steps.extend(make_v_panel(2))
                            if ht + 1 < NP:
                                v_steps.extend(make_proj(ht + 1))
                        QZ, KTc = QK[ht]

                        if prev is not None:
                            po = [pop.tile([128, 512], F32, name=f"po{qb}",
                                           tag=f"po{qb}") for qb in range(2)]
                            prev = (prev[0], prev[1], po)
                        PTl = [ptp.tile([128, NQ], BF16, name=f"PT{hp}_{k}",
                                        tag=f"PT{hp}_{k}")
                               for k in range(NT)]

                        for k in range(NT):
                            psS = psSp.tile([128, NQ], F32, name="psS",
                                            tag="psS")
                            for qb in range(2):
                                qs = slice(qb * 512, (qb + 1) * 512)
                                nc.tensor.matmul(
                                    psS[:, qs],
                                    KTc[k // 4][:, (k % 4) * 128:
                                                 (k % 4 + 1) * 128],
                                    QZ[hp][:, qs],
                                    start=True, stop=True)
                            emit_exp(psS, PTl[k], k)
                            if prev is not None:
                                av_step(prev[0], prev[1], prev[2], k)
                            if hp == 0 and k == 3 and ht >= 2:
                                normalize_pair(ht - 2)
                            pump(1)
                        if prev is not None:
                            av_drain(prev[0], prev[2])
                        prev = (h, PTl, None)

                        if debug_taps and h == 0:
                            nc.sync.dma_start(taps["tap_QT0"].ap(), QZ[0][:])
                            for j in range(4):
                                nc.sync.dma_start(
                                    taps["tap_KT0"].ap()[:, j * 512:
                                                         (j + 1) * 512],
                                    KTc[j][:])
                            nc.sync.dma_start(taps["tap_PTA4"].ap(),
                                              PTl[4][:])
                            nc.sync.dma_start(taps["tap_PTA5"].ap(),
                                              PTl[5][:])

                # ---- epilogue: last half's attn@V, normalizes, out-proj ----
                with tc.tile_pool(name="psC", bufs=4, space="PSUM") as psC:
                    h, PTl, _ = prev
                    po = [pop.tile([128, 512], F32, name=f"po{qb}",
                                   tag=f"po{qb}") for qb in range(2)]
                    for k in range(NT):
                        av_step(h, PTl, po, k)
                    av_drain(h, po)
                    normalize_pair(NP - 2)
                    normalize_pair(NP - 1)

                    groups = [(qt, fo, fsz) for qt in range(QT_TILES)
                              for (fo, fsz) in ((0, 512), (512, 256))]
                    for (qt, fo, fsz) in groups:
                        ps = psC.tile([128, 512], F32, name="psF", tag="psF")
                        for i in range(DT):
                            nc.tensor.matmul(
                                ps[:, :fsz],
                                AOT[i][:, qt * 128:(qt + 1) * 128],
                                WOB[:, i * D + fo:i * D + fo + fsz],
                                start=(i == 0), stop=(i == DT - 1))
                        ot = outsp.tile([128, 512], F32, name="ot", tag="ot")
                        nc.vector.tensor_add(
                            ot[:, :fsz], ps[:, :fsz], BIAS[:, fo:fo + fsz])
                        nc.sync.dma_start(
                            out_d.ap()[qt * 128:(qt + 1) * 128, fo:fo + fsz],
                            ot[:, :fsz])

    nc.compile()
    return nc


_NC = None


def _get_nc():
    global _NC
    if _NC is None:
        _NC = build()
    return _NC


def make_in_maps(x, w_qkv, w_out, b_out):
    import ml_dtypes
    x = np.asarray(x, np.float32)
    w_qkv = np.ascontiguousarray(np.asarray(w_qkv, ml_dtypes.bfloat16))
    w_out = np.ascontiguousarray(np.asarray(w_out, ml_dtypes.bfloat16))
    bias = np.ascontiguousarray(
        np.broadcast_to(np.asarray(b_out, np.float32)[None, :], (128, D)))
    in_maps = []
    for c in range(N_CORES):
        b, half = divmod(c, 2)
        xb = x[b]
        qoff = half * NQ
        # query half first; key order permutation is harmless
        xperm = np.vstack([xb[qoff:qoff + NQ], xb[NQ - qoff:2 * NQ - qoff]])
        in_maps.append({
            "xT": np.ascontiguousarray(xperm.T.astype(ml_dtypes.bfloat16)),
            "wqkv": w_qkv,
            "wout": w_out,
            "bias": bias,
        })
    return in_maps


def run(in_maps, trace=False, **kw):
    return run_bass_kernel_spmd(_get_nc(), in_maps,
                                core_ids=list(range(N_CORES)),
                                trace=trace, **kw)


def assemble(results):
    out = np.empty((B, N, D), np.float32)
    for c in range(N_CORES):
        b, half = divmod(c, 2)
        out[b, half * NQ:(half + 1) * NQ, :] = results[c]["out"]
    return out


def kernel(x, w_qkv, w_out, b_out):
    res = run(make_in_maps(x, w_qkv, w_out, b_out))
    return assemble(res.results)
